# revision 26
# baseline (speedup 1.0000x reference)
"""AIG triple embedding layer on 8 TRN2 NeuronCores.

Math: out[t] = W @ concat(src[t], r[t], dst[t]) + b
            = TA[fs[t]] + TB[fd[t]] + (EW0 + b) + rel[t]*(EW1 - EW0)
where the src/dst node tables are DEDUPED: the reference's input and output
sinusoid tables are identical (same n, d), so the flat table is
  U = [sinusoid(256) | gate[:256]]  (512 rows)
  fs = {0,1}->idx, 2->256+idx, 3->512 (out of range -> zero contribution)
  TA = U @ W1.T, TB = U @ W3.T  (host-precomputed weight algebra, fp16)
  EW0+b goes into the ScalarE evacuation bias; (EW1-EW0)*rel is a rank-3
  selector matmul (linear in rel -- no rel one-hot needed).

Device impl "selmm" (row selection as TensorE matmuls):
  - index math once on DVE in natural [128, 512] layout (t = p*512 + u),
    cast to fp16
  - fs/fd/rel rows for each 8192-triple block are stacked into a [3, 8192]
    SBUF tile at partition 0 via SBUF->SBUF DMA
  - per 512-triple group: two selector matmuls broadcast fs/fd across
    partitions ([128,512] PSUM), ScalarE copies to SBUF fp16, DVE builds
    4+4 one-hot chunks (is_equal vs per-partition iota), and 8 accumulating
    matmuls + 1 rel-selector matmul produce psum[d, t]; ScalarE evacuates
    with the (EW0+b) bias; output stays TRANSPOSED [d, t] in DRAM (fp16)
    and the host transposes/upcasts during unsharding.

Sharding: data-parallel over T across 8 cores; tables/weights replicated.
"""

import numpy as np

D = 128
T = 524288
NCORES = 8
NSHARD = T // NCORES  # 65536
NI = 256              # num_input_nodes == num_output_nodes == IDX_MAX
P = 128
U = NSHARD // P       # 512 triples per partition row
NBLK = 8              # blocks of 16 partition-rows (8192 triples)
GRP = 512             # triples per psum group
NCH = 4               # 128-row chunks in the deduped 512-row table

IMPL = "selmm"

_CACHE = {}


def _sinusoid(n, d):
    pos = np.arange(n, dtype=np.float32)[:, None]
    div = np.exp(np.arange(0, d, 2, dtype=np.float32)
                 * (-np.log(np.float32(10000.0)) / np.float32(d)))
    ang = (pos * div).astype(np.float32)
    enc = np.zeros((n, d), np.float32)
    enc[:, 0::2] = np.sin(ang)
    enc[:, 1::2] = np.cos(ang)
    return enc


def _build_nc_selmm():
    import concourse.bacc as bacc
    import concourse.mybir as mybir
    import concourse.tile as tile

    f32 = mybir.dt.float32
    f16 = mybir.dt.float16
    i32 = mybir.dt.int32
    AL = mybir.AluOpType
    AF = mybir.ActivationFunctionType

    nc = bacc.Bacc(None, target_bir_lowering=False)

    ta = nc.dram_tensor("ta", [NCH * P, D], f16, kind="ExternalInput")
    tb = nc.dram_tensor("tb", [NCH * P, D], f16, kind="ExternalInput")
    selw = nc.dram_tensor("selw", [3, 3 * D], f16, kind="ExternalInput")
    ew0b = nc.dram_tensor("ew0b", [P, 1], f32, kind="ExternalInput")
    ioc = nc.dram_tensor("ioc", [P, NCH], f32, kind="ExternalInput")
    s_i = nc.dram_tensor("src_idx", [NSHARD], i32, kind="ExternalInput")
    s_t = nc.dram_tensor("src_type", [NSHARD], i32, kind="ExternalInput")
    r_l = nc.dram_tensor("rel", [NSHARD], i32, kind="ExternalInput")
    d_i = nc.dram_tensor("dst_idx", [NSHARD], i32, kind="ExternalInput")
    d_t = nc.dram_tensor("dst_type", [NSHARD], i32, kind="ExternalInput")
    outT = nc.dram_tensor("outT", [P, NSHARD], f16, kind="ExternalOutput")

    with tile.TileContext(nc) as tc:
        with (
            tc.tile_pool(name="const", bufs=1) as cpool,
            tc.tile_pool(name="psumB", bufs=2, space="PSUM") as pB,
            tc.tile_pool(name="psumO", bufs=4, space="PSUM") as pO,
            tc.tile_pool(name="bcast", bufs=4) as xpool,
            tc.tile_pool(name="oh", bufs=10) as ohp,
            tc.tile_pool(name="stage", bufs=2) as spool,
            tc.tile_pool(name="outs", bufs=4) as osp,
        ):
            # ---------------- constants ----------------
            TAc = cpool.tile([P, NCH, D], f16)
            nc.sync.dma_start(out=TAc[:],
                              in_=ta[:].rearrange("(c p) d -> p c d", p=P))
            TBc = cpool.tile([P, NCH, D], f16)
            nc.sync.dma_start(out=TBc[:],
                              in_=tb[:].rearrange("(c p) d -> p c d", p=P))
            ioc_sb = cpool.tile([P, NCH], f32)
            nc.sync.dma_start(out=ioc_sb[:], in_=ioc[:])
            ew0b_sb = cpool.tile([P, 1], f32)
            nc.sync.dma_start(out=ew0b_sb[:], in_=ew0b[:])
            # selector weights: [3, 3*128] fp16
            #   cols 0:128   -> ones in row 0 (select fs row)
            #   cols 128:256 -> ones in row 1 (select fd row)
            #   cols 256:384 -> row 2 = EW1-EW0 (rel contribution)
            selt = cpool.tile([3, 3 * P], f16)
            nc.sync.dma_start(out=selt[:], in_=selw[:])

            # ---------------- natural-layout index math ----------------
            sti = cpool.tile([P, U], i32)
            sii = cpool.tile([P, U], i32)
            rli = cpool.tile([P, U], i32)
            dti = cpool.tile([P, U], i32)
            dii = cpool.tile([P, U], i32)
            for tl, h in ((sti, s_t), (sii, s_i), (rli, r_l),
                          (dti, d_t), (dii, d_i)):
                nc.sync.dma_start(out=tl[:],
                                  in_=h[:].rearrange("(p u) -> p u", p=P))

            fs16 = cpool.tile([P, U], f16)
            fd16 = cpool.tile([P, U], f16)
            rl16 = cpool.tile([P, U], f16)
            tmp1 = cpool.tile([P, U], i32)
            tmp2 = cpool.tile([P, U], i32)
            for (ti, ii, o16) in ((sti, sii, fs16), (dti, dii, fd16)):
                # off = max(type-1, 0) << 8  -> {0, 0, 256, 512}
                nc.vector.tensor_scalar(out=tmp1[:], in0=ti[:], scalar1=1,
                                        scalar2=0, op0=AL.subtract, op1=AL.max)
                nc.vector.tensor_scalar(out=tmp1[:], in0=tmp1[:], scalar1=8,
                                        scalar2=None,
                                        op0=AL.logical_shift_left)
                # keep = (type != 3)
                nc.vector.tensor_scalar(out=tmp2[:], in0=ti[:], scalar1=3,
                                        scalar2=None, op0=AL.not_equal)
                nc.vector.tensor_tensor(out=tmp2[:], in0=ii[:], in1=tmp2[:],
                                        op=AL.mult)
                nc.vector.tensor_tensor(out=tmp1[:], in0=tmp1[:], in1=tmp2[:],
                                        op=AL.add)
                nc.vector.tensor_copy(out=o16[:], in_=tmp1[:])
            nc.vector.tensor_copy(out=rl16[:], in_=rli[:])

            # ---------------- main loop ----------------
            for b in range(NBLK):
                # stack fs/fd/rel rows of this block to partitions 0..2
                stage = spool.tile([3, 16, U], f16, tag="stage")
                p0 = b * 16
                for row, src in ((0, fs16), (1, fd16), (2, rl16)):
                    nc.sync.dma_start(out=stage[row:row + 1, :, :],
                                      in_=src[p0:p0 + 16, :])

                stg = stage[:].rearrange("r a u -> r (a u)")
                for g16 in range(16):
                    g = 16 * b + g16
                    sl = stg[:, g16 * GRP:(g16 + 1) * GRP]
                    psF = pB.tile([P, GRP], f32, tag="psF")
                    nc.tensor.matmul(out=psF[:], lhsT=selt[:, 0:P], rhs=sl,
                                     start=True, stop=True)
                    psD = pB.tile([P, GRP], f32, tag="psD")
                    nc.tensor.matmul(out=psD[:], lhsT=selt[:, P:2 * P], rhs=sl,
                                     start=True, stop=True)
                    FSs = xpool.tile([P, GRP], f16, tag="FSs")
                    nc.scalar.activation(FSs[:], psF[:], AF.Copy)
                    FDs = xpool.tile([P, GRP], f16, tag="FDs")
                    nc.scalar.activation(FDs[:], psD[:], AF.Copy)

                    psO = pO.tile([P, GRP], f32, tag="psO")
                    for c in range(NCH):
                        oh = ohp.tile([P, GRP], f16, tag="oh")
                        nc.vector.tensor_scalar(out=oh[:], in0=FSs[:],
                                                scalar1=ioc_sb[:, c:c + 1],
                                                scalar2=None, op0=AL.is_equal)
                        nc.tensor.matmul(out=psO[:], lhsT=TAc[:, c, :],
                                         rhs=oh[:], start=(c == 0),
                                         stop=False)
                    for c in range(NCH):
                        oh = ohp.tile([P, GRP], f16, tag="oh")
                        nc.vector.tensor_scalar(out=oh[:], in0=FDs[:],
                                                scalar1=ioc_sb[:, c:c + 1],
                                                scalar2=None, op0=AL.is_equal)
                        nc.tensor.matmul(out=psO[:], lhsT=TBc[:, c, :],
                                         rhs=oh[:], start=False, stop=False)
                    nc.tensor.matmul(out=psO[:], lhsT=selt[:, 2 * P:3 * P],
                                     rhs=sl, start=False, stop=True)

                    osb = osp.tile([P, GRP], f16, tag="osb")
                    nc.scalar.activation(osb[:], psO[:], AF.Identity,
                                         bias=ew0b_sb[:, 0:1])
                    nc.sync.dma_start(out=outT[:, g * GRP:(g + 1) * GRP],
                                      in_=osb[:])

    nc.compile()
    return nc


def _make_in_maps(inputs):
    gate = np.asarray(inputs["gate_emb"], np.float32)
    edge = np.asarray(inputs["edge_emb"], np.float32)
    W = np.asarray(inputs["W"], np.float32)
    b = np.asarray(inputs["b"], np.float32)

    Utbl = np.concatenate([_sinusoid(NI, D), gate[:NI]], axis=0)  # [512,128]
    W1 = W[:, 0:D]
    W2 = W[:, D:2 * D]
    W3 = W[:, 2 * D:3 * D]
    TA = (Utbl @ W1.T).astype(np.float16)        # [512, 128]
    TB = (Utbl @ W3.T).astype(np.float16)
    ew0b = (edge[0] @ W2.T + b).astype(np.float32).reshape(P, 1)
    ewd = ((edge[1] - edge[0]) @ W2.T).astype(np.float16).reshape(D)
    selw = np.zeros((3, 3 * D), np.float16)
    selw[0, 0:D] = 1.0
    selw[1, D:2 * D] = 1.0
    selw[2, 2 * D:3 * D] = ewd
    ioc = (np.arange(P, dtype=np.float32)[:, None]
           + P * np.arange(NCH, dtype=np.float32)[None, :])

    common = {
        "ta": TA, "tb": TB, "selw": selw, "ew0b": ew0b, "ioc": ioc,
    }
    idx_names = ["src_idx", "src_type", "rel", "dst_idx", "dst_type"]
    idx = {k: np.ascontiguousarray(np.asarray(inputs[k]).astype(np.int32))
           for k in idx_names}

    in_maps = []
    for c in range(NCORES):
        m = dict(common)
        for k in idx_names:
            m[k] = np.ascontiguousarray(idx[k][c * NSHARD:(c + 1) * NSHARD])
        in_maps.append(m)
    return in_maps


def _post(core_result):
    """Device output -> this core's [NSHARD, D] float32 block."""
    return core_result["outT"].T.astype(np.float32)


BUILDERS = {"selmm": _build_nc_selmm}
DEV_OUT = "outT"


def kernel(**inputs):
    from concourse.bass_utils import run_bass_kernel_spmd

    if "nc" not in _CACHE:
        _CACHE["nc"] = BUILDERS[IMPL]()
    nc = _CACHE["nc"]

    in_maps = _make_in_maps(inputs)
    res = run_bass_kernel_spmd(nc, in_maps, core_ids=list(range(NCORES)))
    return np.concatenate([_post(res.results[c]) for c in range(NCORES)],
                          axis=0)


# revision 27
# speedup vs baseline: 1.0263x; 1.0263x over previous
"""AIG triple embedding layer on 8 TRN2 NeuronCores.

Math: out[t] = W @ concat(src[t], r[t], dst[t]) + b
            = TA[fs[t]] + TB[fd[t]] + (EW0 + b) + rel[t]*(EW1 - EW0)
where the src/dst node tables are DEDUPED: the reference's input and output
sinusoid tables are identical (same n, d), so the flat table is
  U = [sinusoid(256) | gate[:256]]  (512 rows)
  fs = {0,1}->idx, 2->256+idx, 3->512 (out of range -> zero contribution)
  TA = U @ W1.T, TB = U @ W3.T  (host-precomputed weight algebra, fp16)
  EW0+b goes into the ScalarE evacuation bias; (EW1-EW0)*rel is a rank-3
  selector matmul (linear in rel -- no rel one-hot needed).

Device impl "selmm" (row selection as TensorE matmuls):
  - index math once on DVE in natural [128, 512] layout (t = p*512 + u),
    cast to fp16
  - fs/fd/rel rows for each 8192-triple block are stacked into a [3, 8192]
    SBUF tile at partition 0 via SBUF->SBUF DMA
  - per 512-triple group: two selector matmuls broadcast fs/fd across
    partitions ([128,512] PSUM), ScalarE copies to SBUF fp16, DVE builds
    4+4 one-hot chunks (is_equal vs per-partition iota), and 8 accumulating
    matmuls + 1 rel-selector matmul produce psum[d, t]; ScalarE evacuates
    with the (EW0+b) bias; output stays TRANSPOSED [d, t] in DRAM (fp16)
    and the host transposes/upcasts during unsharding.

Sharding: data-parallel over T across 8 cores; tables/weights replicated.
"""

import numpy as np

D = 128
T = 524288
NCORES = 8
NSHARD = T // NCORES  # 65536
NI = 256              # num_input_nodes == num_output_nodes == IDX_MAX
P = 128
U = NSHARD // P       # 512 triples per partition row
NBLK = 8              # blocks of 16 partition-rows (8192 triples)
GRP = 512             # triples per psum group
NCH = 4               # 128-row chunks in the deduped 512-row table

IMPL = "selmm"

_CACHE = {}


def _sinusoid(n, d):
    pos = np.arange(n, dtype=np.float32)[:, None]
    div = np.exp(np.arange(0, d, 2, dtype=np.float32)
                 * (-np.log(np.float32(10000.0)) / np.float32(d)))
    ang = (pos * div).astype(np.float32)
    enc = np.zeros((n, d), np.float32)
    enc[:, 0::2] = np.sin(ang)
    enc[:, 1::2] = np.cos(ang)
    return enc


def _build_nc_selmm():
    import concourse.bacc as bacc
    import concourse.mybir as mybir
    import concourse.tile as tile

    f32 = mybir.dt.float32
    f16 = mybir.dt.float16
    i32 = mybir.dt.int32
    AL = mybir.AluOpType
    AF = mybir.ActivationFunctionType

    nc = bacc.Bacc(None, target_bir_lowering=False)

    ta = nc.dram_tensor("ta", [NCH * P, D], f16, kind="ExternalInput")
    tb = nc.dram_tensor("tb", [NCH * P, D], f16, kind="ExternalInput")
    selw = nc.dram_tensor("selw", [3, 3 * D], f16, kind="ExternalInput")
    ew0b = nc.dram_tensor("ew0b", [P, 1], f32, kind="ExternalInput")
    ioc = nc.dram_tensor("ioc", [P, NCH], f32, kind="ExternalInput")
    s_i = nc.dram_tensor("src_idx", [NSHARD], i32, kind="ExternalInput")
    s_t = nc.dram_tensor("src_type", [NSHARD], i32, kind="ExternalInput")
    r_l = nc.dram_tensor("rel", [NSHARD], i32, kind="ExternalInput")
    d_i = nc.dram_tensor("dst_idx", [NSHARD], i32, kind="ExternalInput")
    d_t = nc.dram_tensor("dst_type", [NSHARD], i32, kind="ExternalInput")
    outT = nc.dram_tensor("outT", [P, NSHARD], f16, kind="ExternalOutput")

    with tile.TileContext(nc) as tc:
        with (
            tc.tile_pool(name="const", bufs=1) as cpool,
            tc.tile_pool(name="psumB", bufs=3, space="PSUM") as pB,
            tc.tile_pool(name="psumO", bufs=2, space="PSUM") as pO,
            tc.tile_pool(name="bcast", bufs=4) as xpool,
            tc.tile_pool(name="oh", bufs=10) as ohp,
            tc.tile_pool(name="stage", bufs=2) as spool,
            tc.tile_pool(name="outs", bufs=4) as osp,
        ):
            # ---------------- constants ----------------
            TAc = cpool.tile([P, NCH, D], f16)
            nc.sync.dma_start(out=TAc[:],
                              in_=ta[:].rearrange("(c p) d -> p c d", p=P))
            TBc = cpool.tile([P, NCH, D], f16)
            nc.sync.dma_start(out=TBc[:],
                              in_=tb[:].rearrange("(c p) d -> p c d", p=P))
            ioc_sb = cpool.tile([P, NCH], f32)
            nc.sync.dma_start(out=ioc_sb[:], in_=ioc[:])
            ew0b_sb = cpool.tile([P, 1], f32)
            nc.sync.dma_start(out=ew0b_sb[:], in_=ew0b[:])
            # selector weights: [3, 3*128] fp16
            #   cols 0:128   -> ones in row 0 (select fs row)
            #   cols 128:256 -> ones in row 1 (select fd row)
            #   cols 256:384 -> row 2 = EW1-EW0 (rel contribution)
            selt = cpool.tile([3, 3 * P], f16)
            nc.sync.dma_start(out=selt[:], in_=selw[:])

            # ---------------- natural-layout index math ----------------
            sti = cpool.tile([P, U], i32)
            sii = cpool.tile([P, U], i32)
            rli = cpool.tile([P, U], i32)
            dti = cpool.tile([P, U], i32)
            dii = cpool.tile([P, U], i32)
            for tl, h in ((sti, s_t), (sii, s_i), (rli, r_l),
                          (dti, d_t), (dii, d_i)):
                nc.sync.dma_start(out=tl[:],
                                  in_=h[:].rearrange("(p u) -> p u", p=P))

            fs16 = cpool.tile([P, U], f16)
            fd16 = cpool.tile([P, U], f16)
            rl16 = cpool.tile([P, U], f16)
            tmp1 = cpool.tile([P, U], i32)
            tmp2 = cpool.tile([P, U], i32)
            for (ti, ii, o16) in ((sti, sii, fs16), (dti, dii, fd16)):
                # off = max(type-1, 0) << 8  -> {0, 0, 256, 512}
                nc.vector.tensor_scalar(out=tmp1[:], in0=ti[:], scalar1=1,
                                        scalar2=0, op0=AL.subtract, op1=AL.max)
                nc.vector.tensor_scalar(out=tmp1[:], in0=tmp1[:], scalar1=8,
                                        scalar2=None,
                                        op0=AL.logical_shift_left)
                # keep = (type != 3)
                nc.vector.tensor_scalar(out=tmp2[:], in0=ti[:], scalar1=3,
                                        scalar2=None, op0=AL.not_equal)
                nc.vector.tensor_tensor(out=tmp2[:], in0=ii[:], in1=tmp2[:],
                                        op=AL.mult)
                nc.vector.tensor_tensor(out=tmp1[:], in0=tmp1[:], in1=tmp2[:],
                                        op=AL.add)
                nc.vector.tensor_copy(out=o16[:], in_=tmp1[:])
            nc.vector.tensor_copy(out=rl16[:], in_=rli[:])

            # ---------------- main loop ----------------
            for b in range(NBLK):
                # stack fs/fd/rel rows of this block to partitions 0..2
                stage = spool.tile([3, 16, U], f16, tag="stage")
                p0 = b * 16
                for row, src in ((0, fs16), (1, fd16), (2, rl16)):
                    nc.sync.dma_start(out=stage[row:row + 1, :, :],
                                      in_=src[p0:p0 + 16, :])

                stg = stage[:].rearrange("r a u -> r (a u)")
                for g16 in range(16):
                    g = 16 * b + g16
                    sl = stg[:, g16 * GRP:(g16 + 1) * GRP]
                    psF = pB.tile([P, GRP], f32, tag="psF")
                    nc.tensor.matmul(out=psF[:], lhsT=selt[:, 0:P], rhs=sl,
                                     start=True, stop=True)
                    psD = pB.tile([P, GRP], f32, tag="psD")
                    nc.tensor.matmul(out=psD[:], lhsT=selt[:, P:2 * P], rhs=sl,
                                     start=True, stop=True)
                    FSs = xpool.tile([P, GRP], f16, tag="FSs")
                    nc.scalar.activation(FSs[:], psF[:], AF.Copy)
                    FDs = xpool.tile([P, GRP], f16, tag="FDs")
                    nc.scalar.activation(FDs[:], psD[:], AF.Copy)

                    psO = pO.tile([P, GRP], f32, tag="psO")
                    for c in range(NCH):
                        oh = ohp.tile([P, GRP], f16, tag="oh")
                        nc.vector.tensor_scalar(out=oh[:], in0=FSs[:],
                                                scalar1=ioc_sb[:, c:c + 1],
                                                scalar2=None, op0=AL.is_equal)
                        nc.tensor.matmul(out=psO[:], lhsT=TAc[:, c, :],
                                         rhs=oh[:], start=(c == 0),
                                         stop=False)
                    for c in range(NCH):
                        oh = ohp.tile([P, GRP], f16, tag="oh")
                        nc.vector.tensor_scalar(out=oh[:], in0=FDs[:],
                                                scalar1=ioc_sb[:, c:c + 1],
                                                scalar2=None, op0=AL.is_equal)
                        nc.tensor.matmul(out=psO[:], lhsT=TBc[:, c, :],
                                         rhs=oh[:], start=False, stop=False)
                    nc.tensor.matmul(out=psO[:], lhsT=selt[:, 2 * P:3 * P],
                                     rhs=sl, start=False, stop=True)

                    osb = osp.tile([P, GRP], f16, tag="osb")
                    nc.scalar.activation(osb[:], psO[:], AF.Identity,
                                         bias=ew0b_sb[:, 0:1])
                    nc.sync.dma_start(out=outT[:, g * GRP:(g + 1) * GRP],
                                      in_=osb[:])

    nc.compile()
    return nc


def _make_in_maps(inputs):
    gate = np.asarray(inputs["gate_emb"], np.float32)
    edge = np.asarray(inputs["edge_emb"], np.float32)
    W = np.asarray(inputs["W"], np.float32)
    b = np.asarray(inputs["b"], np.float32)

    Utbl = np.concatenate([_sinusoid(NI, D), gate[:NI]], axis=0)  # [512,128]
    W1 = W[:, 0:D]
    W2 = W[:, D:2 * D]
    W3 = W[:, 2 * D:3 * D]
    TA = (Utbl @ W1.T).astype(np.float16)        # [512, 128]
    TB = (Utbl @ W3.T).astype(np.float16)
    ew0b = (edge[0] @ W2.T + b).astype(np.float32).reshape(P, 1)
    ewd = ((edge[1] - edge[0]) @ W2.T).astype(np.float16).reshape(D)
    selw = np.zeros((3, 3 * D), np.float16)
    selw[0, 0:D] = 1.0
    selw[1, D:2 * D] = 1.0
    selw[2, 2 * D:3 * D] = ewd
    ioc = (np.arange(P, dtype=np.float32)[:, None]
           + P * np.arange(NCH, dtype=np.float32)[None, :])

    common = {
        "ta": TA, "tb": TB, "selw": selw, "ew0b": ew0b, "ioc": ioc,
    }
    idx_names = ["src_idx", "src_type", "rel", "dst_idx", "dst_type"]
    idx = {k: np.ascontiguousarray(np.asarray(inputs[k]).astype(np.int32))
           for k in idx_names}

    in_maps = []
    for c in range(NCORES):
        m = dict(common)
        for k in idx_names:
            m[k] = np.ascontiguousarray(idx[k][c * NSHARD:(c + 1) * NSHARD])
        in_maps.append(m)
    return in_maps


def _post(core_result):
    """Device output -> this core's [NSHARD, D] float32 block."""
    return core_result["outT"].T.astype(np.float32)


BUILDERS = {"selmm": _build_nc_selmm}
DEV_OUT = "outT"


def kernel(**inputs):
    from concourse.bass_utils import run_bass_kernel_spmd

    if "nc" not in _CACHE:
        _CACHE["nc"] = BUILDERS[IMPL]()
    nc = _CACHE["nc"]

    in_maps = _make_in_maps(inputs)
    res = run_bass_kernel_spmd(nc, in_maps, core_ids=list(range(NCORES)))
    return np.concatenate([_post(res.results[c]) for c in range(NCORES)],
                          axis=0)
